# revision 24
# baseline (speedup 1.0000x reference)
"""MinGRU Trainium2 kernel.

Problem: B=8, T=4096, D=512, H=512 MinGRU:
    k = x @ Wz^T + bz;  z = sigmoid(k)
    w = x @ Wh^T + bh;  h~ = g(w),  g(w) = relu(w) + 0.5 (w>=0) | sigmoid(w) (w<0)
    h_t = (1 - z_t) * h_{t-1} + z_t * h~_t,   h_{-1} = g(h_0)
(The reference computes this recurrence in log space via cumlogsumexp; in
linear space all quantities are positive and bounded, so a direct scan with
fp32 state is numerically stable.)

Sharding: data-parallel over batch, one batch row per NeuronCore (8 cores).

Per-core device layout (everything transposed so H sits on partitions and T
on the free dim, which lets the VectorE `tensor_tensor_scan` instruction run
the recurrence along T):
    xT  (D=512, T=4096)  f32r  - host pre-transposed
    wzT/whT (D=512, H=512) f32r - host pre-transposed weights (lhsT layout)
    k^T/w^T tiles computed on PE in PSUM with float32r (full-rate fp32)
    a    = sigmoid(-k - bz)                      [ScalarE, bias/scale fused]
    s    = sigmoid(w + bh)                       [ScalarE]
    r1   = relu(w + bh)                          [ScalarE]
    g    = min(s, 0.5) + r1                      [VectorE scalar_tensor_tensor]
           (identity: sigmoid(min(v,0)) = min(sigmoid(v), 0.5))
    bneg = (a - 1) * g                           [VectorE scalar_tensor_tensor]
    h    = scan: state = a*state - bneg          [VectorE tensor_tensor_scan,
                                                  fp32 internal state]
    hT out (H=512, T=4096) -> host transposes back

The elementwise chain runs in bf16 (DVE 2x packed mode); matmuls and the
scan state stay fp32.
"""

import os

import numpy as np

import concourse.bass as bass
import concourse.mybir as mybir
import concourse.tile as tile
from concourse import bacc
from concourse.bass_utils import run_bass_kernel_spmd

# Problem constants (hardcoded per harness contract).
B, T, D, H = 8, 4096, 512, 512
P = 128          # partitions
DB = D // P      # 4 contraction blocks
HB = H // P      # 4 output h blocks
TC = 2048        # T chunk per elementwise tile
NT = T // TC     # 2
MM_N = 512       # matmul free-dim chunk
NCC = TC // MM_N # 4 matmul column chunks per tile

F32 = mybir.dt.float32
F32R = mybir.dt.float32r
BF16 = mybir.dt.bfloat16
EW = BF16        # elementwise chain dtype

# Stash of the last run's BassKernelResults (for test harness introspection).
LAST_RESULT = None


def _build_nc():
    nc = bacc.Bacc(
        "TRN2",
        target_bir_lowering=False,
        debug=False,
        enable_asserts=False,
        num_devices=B,
    )

    xT_d = nc.dram_tensor("xT", (D, T), BF16, kind="ExternalInput")
    # wT layout: (DB, P, H) so one DMA lands all four 128-row blocks side by
    # side in a single (P, DB*H) SBUF tile.
    wzT_d = nc.dram_tensor("wzT", (DB, P, H), BF16, kind="ExternalInput")
    whT_d = nc.dram_tensor("whT", (DB, P, H), BF16, kind="ExternalInput")
    # smalls columns: [0:4] -bz per h-block, [4:8] bh, [8:12] g(h_0) carries
    smalls_d = nc.dram_tensor("smalls", (P, 16), F32, kind="ExternalInput")
    hT_d = nc.dram_tensor("hT", (H, T), EW, kind="ExternalOutput")

    AF = mybir.ActivationFunctionType
    OP = mybir.AluOpType

    from contextlib import ExitStack

    with tile.TileContext(nc) as tc, ExitStack() as ctx:
        wpool = ctx.enter_context(tc.tile_pool(name="weights", bufs=1))
        xpool = ctx.enter_context(tc.tile_pool(name="xtiles", bufs=2 * DB))
        spool = ctx.enter_context(tc.tile_pool(name="work", bufs=3))
        ppool = ctx.enter_context(tc.tile_pool(name="psum", bufs=2, space="PSUM"))

        # --- Setup DMAs: first x tile, then wz (first matmul needs both),
        # then the rest; tiny smalls on the gpsimd SWDGE ring in parallel.
        smalls = wpool.tile([P, 16], F32, name="smalls")
        nc.gpsimd.dma_start(smalls[:], smalls_d.ap()[:])

        wz_sb = wpool.tile([P, DB * H], BF16, name="wz_sb")
        wh_sb = wpool.tile([P, DB * H], BF16, name="wh_sb")
        # weights on the scalar HWDGE ring: their descriptor generation runs
        # in parallel with the x-tile DMAs on the sync ring (descriptor-gen
        # serializes at ~0.6us/DMA within one ring).
        xt0 = []
        for db in range(DB):
            x_t = xpool.tile([P, 1024], BF16, name="xt", tag="xt")
            nc.sync.dma_start(x_t[:], xT_d.ap()[db * P:(db + 1) * P, 0:1024])
            xt0.append(x_t)
        for db in range(DB):
            nc.scalar.dma_start(wz_sb[:, db * H:(db + 1) * H], wzT_d.ap()[db])
        for db in range(DB):
            nc.scalar.dma_start(wh_sb[:, db * H:(db + 1) * H], whT_d.ap()[db])

        # PE p-state warmup: stream dummy matmuls on zeroed tiles while the
        # setup DMAs are in flight so the clock is at 2.4GHz when real
        # matmuls start (cold PE runs at ~1/3 speed for the first ~3us).
        dwa = wpool.tile([P, 128], BF16, name="dwa")
        nc.gpsimd.memset(dwa[:], 0.0)
        dwb = wpool.tile([P, 512], BF16, name="dwb")
        nc.gpsimd.memset(dwb[:], 0.0)
        for _ in range(20):
            dp = ppool.tile([P, 512], F32, name="dp", tag="kp")
            nc.tensor.matmul(dp[:], dwa[:], dwb[:], start=True, stop=True)

        def wslice(w_sb, db, hb):
            return w_sb[:, db * H + hb * P: db * H + (hb + 1) * P]

        # --- Main loop over T chunks (small first chunk primes the
        # pipeline early; small last chunk shortens the serial tail) ---
        CHUNKS = [1024, 1024, 1024, 1024]
        assert sum(CHUNKS) == T
        starts = [sum(CHUNKS[:i]) for i in range(len(CHUNKS))]

        # first chunk's x tiles (already interleaved with weight DMAs above
        # for chunk 0 -- re-issue here per chunk)
        xt_cur = xt0
        for ci, (ts0, clen) in enumerate(zip(starts, CHUNKS)):
            xt = xt_cur
            if ci + 1 < len(CHUNKS):
                nts0, nclen = starts[ci + 1], CHUNKS[ci + 1]
                xt_nxt = []
                for db in range(DB):
                    x_t = xpool.tile([P, 1024], BF16, name="xt", tag="xt")
                    nc.sync.dma_start(
                        x_t[:, :nclen],
                        xT_d.ap()[db * P:(db + 1) * P, nts0:nts0 + nclen],
                    )
                    xt_nxt.append(x_t)

            ncc = clen // MM_N
            for hb in range(HB):
                hs = slice(hb * P, (hb + 1) * P)
                col = ci * HB + hb
                fast = False  # fast-path disabled: its DVE op overhead outweighed the start gain
                use_z = not fast
                nsub = 2 if fast else 1
                sub = clen // nsub

                kp = ppool.tile([P, 1024], F32, name="kp", tag="kp")
                if fast:
                    for cc in range(ncc):
                        cs = slice(cc * MM_N, (cc + 1) * MM_N)
                        for db in range(DB):
                            nc.tensor.matmul(
                                kp[:, cs], wslice(wz_sb, db, hb),
                                xt[db][:, cs],
                                start=(db == 0), stop=(db == DB - 1),
                            )
                else:
                    for db in range(DB):
                        for cc in range(ncc):
                            cs = slice(cc * MM_N, (cc + 1) * MM_N)
                            nc.tensor.matmul(
                                kp[:, cs], wslice(wz_sb, db, hb),
                                xt[db][:, cs],
                                start=(db == 0), stop=(db == DB - 1),
                            )

                a_t = spool.tile([P, 1024], EW, name="a_t", tag="a")
                for u in range(nsub):
                    us = slice(u * sub, (u + 1) * sub)
                    nc.scalar.activation(
                        a_t[:, us], kp[:, us], AF.Sigmoid,
                        bias=smalls[:, hb:hb + 1], scale=-1.0,
                    )
                if use_z:
                    z_t = spool.tile([P, 1024], EW, name="z_t", tag="z")
                    nc.scalar.activation(
                        z_t[:, :clen], kp[:, :clen], AF.Sigmoid,
                        bias=smalls[:, 12 + hb:13 + hb], scale=1.0,
                    )

                wp = ppool.tile([P, 1024], F32, name="wp", tag="wp")
                if fast:
                    for cc in range(ncc):
                        cs = slice(cc * MM_N, (cc + 1) * MM_N)
                        for db in range(DB):
                            nc.tensor.matmul(
                                wp[:, cs], wslice(wh_sb, db, hb),
                                xt[db][:, cs],
                                start=(db == 0), stop=(db == DB - 1),
                            )
                else:
                    for db in range(DB):
                        for cc in range(ncc):
                            cs = slice(cc * MM_N, (cc + 1) * MM_N)
                            nc.tensor.matmul(
                                wp[:, cs], wslice(wh_sb, db, hb),
                                xt[db][:, cs],
                                start=(db == 0), stop=(db == DB - 1),
                            )

                s_t = spool.tile([P, 1024], EW, name="s_t", tag="s")
                r_t = spool.tile([P, 1024], EW, name="r_t", tag="r")
                for u in range(nsub):
                    us = slice(u * sub, (u + 1) * sub)
                    nc.scalar.activation(
                        s_t[:, us], wp[:, us], AF.Sigmoid,
                        bias=smalls[:, 4 + hb:5 + hb], scale=1.0,
                    )
                    nc.scalar.activation(
                        r_t[:, us], wp[:, us], AF.Relu,
                        bias=smalls[:, 4 + hb:5 + hb], scale=1.0,
                    )

                m_t = spool.tile([P, 1024], EW, name="m_t", tag="m")
                g_t = spool.tile([P, 1024], EW, name="g_t", tag="g")
                bn_t = spool.tile([P, 1024], EW, name="bn_t", tag="bn")
                if not use_z:
                    t_t = spool.tile([P, 1024], EW, name="t_t", tag="t")
                for u in range(nsub):
                    us = slice(u * sub, (u + 1) * sub)
                    nc.vector.tensor_scalar_min(m_t[:, us], s_t[:, us], 0.5)
                    nc.vector.tensor_add(g_t[:, us], m_t[:, us], r_t[:, us])
                    if use_z:
                        nc.vector.tensor_mul(bn_t[:, us], z_t[:, us], g_t[:, us])
                    else:
                        nc.vector.tensor_scalar_sub(t_t[:, us], a_t[:, us], 1.0)
                        nc.vector.tensor_mul(bn_t[:, us], t_t[:, us], g_t[:, us])

                h_t = spool.tile([P, 1024], EW, name="h_t", tag="h")
                op1 = OP.add if use_z else OP.subtract
                last_tile = (ci == len(CHUNKS) - 1) and (hb == HB - 1)
                nscan = 4 if last_tile else (2 if fast else 1)
                ssub = clen // nscan
                for u in range(nscan):
                    us = slice(u * ssub, (u + 1) * ssub)
                    init = (smalls[:, 8 + hb:9 + hb] if u == 0
                            else h_t[:, u * ssub - 1:u * ssub])
                    nc.vector.tensor_tensor_scan(
                        h_t[:, us], a_t[:, us], bn_t[:, us], init,
                        op0=OP.mult, op1=op1,
                    )
                    nc.sync.dma_start(
                        hT_d.ap()[hs, ts0 + u * ssub:ts0 + (u + 1) * ssub],
                        h_t[:, us],
                    )
                if ci + 1 < len(CHUNKS):
                    nc.vector.tensor_copy(
                        smalls[:, 8 + hb:9 + hb], h_t[:, clen - 1:clen]
                    )

            if ci + 1 < len(CHUNKS):
                xt_cur = xt_nxt

    nc.compile()
    return nc


def _host_prep(x, h_0, Wz, bz, Wh, bh):
    x = np.asarray(x, dtype=np.float32)
    h_0 = np.asarray(h_0, dtype=np.float32)
    Wz = np.asarray(Wz, dtype=np.float32)
    bz = np.asarray(bz, dtype=np.float32)
    Wh = np.asarray(Wh, dtype=np.float32)
    bh = np.asarray(bh, dtype=np.float32)

    import ml_dtypes
    bf16 = ml_dtypes.bfloat16
    xT = np.ascontiguousarray(np.transpose(x, (0, 2, 1)).astype(bf16))  # (B, D, T)
    wzT = np.ascontiguousarray(Wz.T.reshape(DB, P, H).astype(bf16))  # (DB, P, H)
    whT = np.ascontiguousarray(Wh.T.reshape(DB, P, H).astype(bf16))

    # initial carry: g(h_0) = min(sigmoid(h_0), 0.5) + relu(h_0)
    sig = 1.0 / (1.0 + np.exp(-h_0.astype(np.float64)))
    h0g = (np.minimum(sig, 0.5) + np.maximum(h_0, 0.0)).astype(np.float32)

    smalls = np.zeros((B, P, 16), dtype=np.float32)
    for hb in range(HB):
        blk = slice(hb * P, (hb + 1) * P)
        smalls[:, :, hb] = -bz[blk]
        smalls[:, :, 4 + hb] = bh[blk]
        smalls[:, :, 8 + hb] = h0g[:, blk]
        smalls[:, :, 12 + hb] = bz[blk]
    smalls = np.ascontiguousarray(smalls)

    in_maps = []
    for i in range(B):
        in_maps.append({
            "xT": xT[i],
            "wzT": wzT,
            "whT": whT,
            "smalls": smalls[i],
        })
    return in_maps


def kernel(x, h_0, Wz, bz, Wh, bh):
    global LAST_RESULT
    in_maps = _host_prep(x, h_0, Wz, bz, Wh, bh)
    nc = _build_nc()
    res = run_bass_kernel_spmd(
        nc,
        in_maps,
        core_ids=list(range(B)),
        trace=bool(int(os.environ.get("MINGRU_TRACE", "0"))),
    )
    LAST_RESULT = res
    out = np.empty((B, T, H), dtype=np.float32)
    for i in range(B):
        out[i] = np.asarray(res.results[i]["hT"]).astype(np.float32).T
    return out


# revision 26
# speedup vs baseline: 1.0109x; 1.0109x over previous
"""MinGRU Trainium2 kernel.

Problem: B=8, T=4096, D=512, H=512 MinGRU:
    k = x @ Wz^T + bz;  z = sigmoid(k)
    w = x @ Wh^T + bh;  h~ = g(w),  g(w) = relu(w) + 0.5 (w>=0) | sigmoid(w) (w<0)
    h_t = (1 - z_t) * h_{t-1} + z_t * h~_t,   h_{-1} = g(h_0)
(The reference computes this recurrence in log space via cumlogsumexp; in
linear space all quantities are positive and bounded, so a direct scan with
fp32 state is numerically stable.)

Sharding: data-parallel over batch, one batch row per NeuronCore (8 cores).

Per-core device layout (everything transposed so H sits on partitions and T
on the free dim, which lets the VectorE `tensor_tensor_scan` instruction run
the recurrence along T):
    xT  (D=512, T=4096)  f32r  - host pre-transposed
    wzT/whT (D=512, H=512) f32r - host pre-transposed weights (lhsT layout)
    k^T/w^T tiles computed on PE in PSUM with float32r (full-rate fp32)
    a    = sigmoid(-k - bz)                      [ScalarE, bias/scale fused]
    s    = sigmoid(w + bh)                       [ScalarE]
    r1   = relu(w + bh)                          [ScalarE]
    g    = min(s, 0.5) + r1                      [VectorE scalar_tensor_tensor]
           (identity: sigmoid(min(v,0)) = min(sigmoid(v), 0.5))
    bneg = (a - 1) * g                           [VectorE scalar_tensor_tensor]
    h    = scan: state = a*state - bneg          [VectorE tensor_tensor_scan,
                                                  fp32 internal state]
    hT out (H=512, T=4096) -> host transposes back

The elementwise chain runs in bf16 (DVE 2x packed mode); matmuls and the
scan state stay fp32.
"""

import os

import numpy as np

import concourse.bass as bass
import concourse.mybir as mybir
import concourse.tile as tile
from concourse import bacc
from concourse.bass_utils import run_bass_kernel_spmd

# Problem constants (hardcoded per harness contract).
B, T, D, H = 8, 4096, 512, 512
P = 128          # partitions
DB = D // P      # 4 contraction blocks
HB = H // P      # 4 output h blocks
TC = 2048        # T chunk per elementwise tile
NT = T // TC     # 2
MM_N = 512       # matmul free-dim chunk
NCC = TC // MM_N # 4 matmul column chunks per tile

F32 = mybir.dt.float32
F32R = mybir.dt.float32r
BF16 = mybir.dt.bfloat16
EW = BF16        # elementwise chain dtype

# Stash of the last run's BassKernelResults (for test harness introspection).
LAST_RESULT = None


def _build_nc():
    nc = bacc.Bacc(
        "TRN2",
        target_bir_lowering=False,
        debug=False,
        enable_asserts=False,
        num_devices=B,
    )

    xT_d = nc.dram_tensor("xT", (D, T), BF16, kind="ExternalInput")
    # wT layout: (DB, P, H) so one DMA lands all four 128-row blocks side by
    # side in a single (P, DB*H) SBUF tile.
    wzT_d = nc.dram_tensor("wzT", (DB, P, H), BF16, kind="ExternalInput")
    whT_d = nc.dram_tensor("whT", (DB, P, H), BF16, kind="ExternalInput")
    # smalls columns: [0:4] -bz per h-block, [4:8] bh, [8:12] g(h_0) carries
    smalls_d = nc.dram_tensor("smalls", (P, 16), F32, kind="ExternalInput")
    hT_d = nc.dram_tensor("hT", (H, T), EW, kind="ExternalOutput")

    AF = mybir.ActivationFunctionType
    OP = mybir.AluOpType

    from contextlib import ExitStack

    with tile.TileContext(nc) as tc, ExitStack() as ctx:
        wpool = ctx.enter_context(tc.tile_pool(name="weights", bufs=1))
        xpool = ctx.enter_context(tc.tile_pool(name="xtiles", bufs=3 * DB))
        spool = ctx.enter_context(tc.tile_pool(name="work", bufs=4))
        ppool = ctx.enter_context(tc.tile_pool(name="psum", bufs=2, space="PSUM"))

        # --- Setup DMAs: first x tile, then wz (first matmul needs both),
        # then the rest; tiny smalls on the gpsimd SWDGE ring in parallel.
        smalls = wpool.tile([P, 16], F32, name="smalls")
        nc.gpsimd.dma_start(smalls[:], smalls_d.ap()[:])

        wz_sb = wpool.tile([P, DB * H], BF16, name="wz_sb")
        wh_sb = wpool.tile([P, DB * H], BF16, name="wh_sb")
        xt0 = []
        for db in range(DB):
            x_t = xpool.tile([P, 1024], BF16, name="xt", tag="xt")
            nc.sync.dma_start(x_t[:, :512], xT_d.ap()[db * P:(db + 1) * P, 0:512])
            xt0.append(x_t)
        # weights on the scalar HWDGE ring: their descriptor generation runs
        # in parallel with the x-tile DMAs on the sync ring (descriptor-gen
        # serializes at ~0.6us/DMA within one ring).
        for db in range(DB):
            nc.scalar.dma_start(wz_sb[:, db * H:(db + 1) * H], wzT_d.ap()[db])
        for db in range(DB):
            nc.sync.dma_start(
                xt0[db][:, 512:1024], xT_d.ap()[db * P:(db + 1) * P, 512:1024]
            )
        for db in range(DB):
            nc.scalar.dma_start(wh_sb[:, db * H:(db + 1) * H], whT_d.ap()[db])

        # PE p-state warmup: stream dummy matmuls on zeroed tiles while the
        # setup DMAs are in flight so the clock is at 2.4GHz when real
        # matmuls start (cold PE runs at ~1/3 speed for the first ~3us).
        dwa = wpool.tile([P, 128], BF16, name="dwa")
        nc.gpsimd.memset(dwa[:], 0.0)
        dwb = wpool.tile([P, 512], BF16, name="dwb")
        nc.gpsimd.memset(dwb[:], 0.0)
        for _ in range(20):
            dp = ppool.tile([P, 512], F32, name="dp", tag="kp")
            nc.tensor.matmul(dp[:], dwa[:], dwb[:], start=True, stop=True)

        def wslice(w_sb, db, hb):
            return w_sb[:, db * H + hb * P: db * H + (hb + 1) * P]

        # --- Main loop over T chunks (small first chunk primes the
        # pipeline early; small last chunk shortens the serial tail) ---
        CHUNKS = [1024, 1024, 1024, 1024]
        assert sum(CHUNKS) == T
        starts = [sum(CHUNKS[:i]) for i in range(len(CHUNKS))]

        # first chunk's x tiles (already interleaved with weight DMAs above
        # for chunk 0 -- re-issue here per chunk)
        xt_cur = xt0
        for ci, (ts0, clen) in enumerate(zip(starts, CHUNKS)):
            xt = xt_cur
            if ci + 1 < len(CHUNKS):
                nts0, nclen = starts[ci + 1], CHUNKS[ci + 1]
                xt_nxt = []
                for db in range(DB):
                    x_t = xpool.tile([P, 1024], BF16, name="xt", tag="xt")
                    nc.sync.dma_start(
                        x_t[:, :nclen],
                        xT_d.ap()[db * P:(db + 1) * P, nts0:nts0 + nclen],
                    )
                    xt_nxt.append(x_t)

            ncc = clen // MM_N
            for hb in range(HB):
                hs = slice(hb * P, (hb + 1) * P)
                col = ci * HB + hb
                fast = False  # fast-path disabled: its DVE op overhead outweighed the start gain
                use_z = not fast
                nsub = 2 if fast else 1
                sub = clen // nsub

                kp = ppool.tile([P, 1024], F32, name="kp", tag="kp")
                if fast:
                    for cc in range(ncc):
                        cs = slice(cc * MM_N, (cc + 1) * MM_N)
                        for db in range(DB):
                            nc.tensor.matmul(
                                kp[:, cs], wslice(wz_sb, db, hb),
                                xt[db][:, cs],
                                start=(db == 0), stop=(db == DB - 1),
                            )
                else:
                    for db in range(DB):
                        for cc in range(ncc):
                            cs = slice(cc * MM_N, (cc + 1) * MM_N)
                            nc.tensor.matmul(
                                kp[:, cs], wslice(wz_sb, db, hb),
                                xt[db][:, cs],
                                start=(db == 0), stop=(db == DB - 1),
                            )

                a_t = spool.tile([P, 1024], EW, name="a_t", tag="a")
                for u in range(nsub):
                    us = slice(u * sub, (u + 1) * sub)
                    nc.scalar.activation(
                        a_t[:, us], kp[:, us], AF.Sigmoid,
                        bias=smalls[:, hb:hb + 1], scale=-1.0,
                    )
                if use_z:
                    z_t = spool.tile([P, 1024], EW, name="z_t", tag="z")
                    nc.scalar.activation(
                        z_t[:, :clen], kp[:, :clen], AF.Sigmoid,
                        bias=smalls[:, 12 + hb:13 + hb], scale=1.0,
                    )

                wp = ppool.tile([P, 1024], F32, name="wp", tag="wp")
                if fast:
                    for cc in range(ncc):
                        cs = slice(cc * MM_N, (cc + 1) * MM_N)
                        for db in range(DB):
                            nc.tensor.matmul(
                                wp[:, cs], wslice(wh_sb, db, hb),
                                xt[db][:, cs],
                                start=(db == 0), stop=(db == DB - 1),
                            )
                else:
                    for db in range(DB):
                        for cc in range(ncc):
                            cs = slice(cc * MM_N, (cc + 1) * MM_N)
                            nc.tensor.matmul(
                                wp[:, cs], wslice(wh_sb, db, hb),
                                xt[db][:, cs],
                                start=(db == 0), stop=(db == DB - 1),
                            )

                s_t = spool.tile([P, 1024], EW, name="s_t", tag="s")
                r_t = spool.tile([P, 1024], EW, name="r_t", tag="r")
                for u in range(nsub):
                    us = slice(u * sub, (u + 1) * sub)
                    nc.scalar.activation(
                        s_t[:, us], wp[:, us], AF.Sigmoid,
                        bias=smalls[:, 4 + hb:5 + hb], scale=1.0,
                    )
                    nc.scalar.activation(
                        r_t[:, us], wp[:, us], AF.Relu,
                        bias=smalls[:, 4 + hb:5 + hb], scale=1.0,
                    )

                m_t = spool.tile([P, 1024], EW, name="m_t", tag="m")
                g_t = spool.tile([P, 1024], EW, name="g_t", tag="g")
                bn_t = spool.tile([P, 1024], EW, name="bn_t", tag="bn")
                if not use_z:
                    t_t = spool.tile([P, 1024], EW, name="t_t", tag="t")
                for u in range(nsub):
                    us = slice(u * sub, (u + 1) * sub)
                    nc.vector.tensor_scalar_min(m_t[:, us], s_t[:, us], 0.5)
                    nc.vector.tensor_add(g_t[:, us], m_t[:, us], r_t[:, us])
                    if use_z:
                        nc.vector.tensor_mul(bn_t[:, us], z_t[:, us], g_t[:, us])
                    else:
                        nc.vector.tensor_scalar_sub(t_t[:, us], a_t[:, us], 1.0)
                        nc.vector.tensor_mul(bn_t[:, us], t_t[:, us], g_t[:, us])

                h_t = spool.tile([P, 1024], EW, name="h_t", tag="h")
                op1 = OP.add if use_z else OP.subtract
                last_tile = (ci == len(CHUNKS) - 1) and (hb == HB - 1)
                nscan = 4 if last_tile else (2 if fast else 1)
                ssub = clen // nscan
                for u in range(nscan):
                    us = slice(u * ssub, (u + 1) * ssub)
                    init = (smalls[:, 8 + hb:9 + hb] if u == 0
                            else h_t[:, u * ssub - 1:u * ssub])
                    nc.vector.tensor_tensor_scan(
                        h_t[:, us], a_t[:, us], bn_t[:, us], init,
                        op0=OP.mult, op1=op1,
                    )
                    nc.sync.dma_start(
                        hT_d.ap()[hs, ts0 + u * ssub:ts0 + (u + 1) * ssub],
                        h_t[:, us],
                    )
                if ci + 1 < len(CHUNKS):
                    nc.vector.tensor_copy(
                        smalls[:, 8 + hb:9 + hb], h_t[:, clen - 1:clen]
                    )

            if ci + 1 < len(CHUNKS):
                xt_cur = xt_nxt

    nc.compile()
    return nc


def _host_prep(x, h_0, Wz, bz, Wh, bh):
    x = np.asarray(x, dtype=np.float32)
    h_0 = np.asarray(h_0, dtype=np.float32)
    Wz = np.asarray(Wz, dtype=np.float32)
    bz = np.asarray(bz, dtype=np.float32)
    Wh = np.asarray(Wh, dtype=np.float32)
    bh = np.asarray(bh, dtype=np.float32)

    import ml_dtypes
    bf16 = ml_dtypes.bfloat16
    xT = np.ascontiguousarray(np.transpose(x, (0, 2, 1)).astype(bf16))  # (B, D, T)
    wzT = np.ascontiguousarray(Wz.T.reshape(DB, P, H).astype(bf16))  # (DB, P, H)
    whT = np.ascontiguousarray(Wh.T.reshape(DB, P, H).astype(bf16))

    # initial carry: g(h_0) = min(sigmoid(h_0), 0.5) + relu(h_0)
    sig = 1.0 / (1.0 + np.exp(-h_0.astype(np.float64)))
    h0g = (np.minimum(sig, 0.5) + np.maximum(h_0, 0.0)).astype(np.float32)

    smalls = np.zeros((B, P, 16), dtype=np.float32)
    for hb in range(HB):
        blk = slice(hb * P, (hb + 1) * P)
        smalls[:, :, hb] = -bz[blk]
        smalls[:, :, 4 + hb] = bh[blk]
        smalls[:, :, 8 + hb] = h0g[:, blk]
        smalls[:, :, 12 + hb] = bz[blk]
    smalls = np.ascontiguousarray(smalls)

    in_maps = []
    for i in range(B):
        in_maps.append({
            "xT": xT[i],
            "wzT": wzT,
            "whT": whT,
            "smalls": smalls[i],
        })
    return in_maps


def kernel(x, h_0, Wz, bz, Wh, bh):
    global LAST_RESULT
    in_maps = _host_prep(x, h_0, Wz, bz, Wh, bh)
    nc = _build_nc()
    res = run_bass_kernel_spmd(
        nc,
        in_maps,
        core_ids=list(range(B)),
        trace=bool(int(os.environ.get("MINGRU_TRACE", "0"))),
    )
    LAST_RESULT = res
    out = np.empty((B, T, H), dtype=np.float32)
    for i in range(B):
        out[i] = np.asarray(res.results[i]["hT"]).astype(np.float32).T
    return out
